# revision 6
# baseline (speedup 1.0000x reference)
"""Bass/Trainium2 kernel for the decomposed LocallyConnected2d layer.

out[b,o,i,j] = sum_{c,k} x[b, c, i+di, j+dj] * w[o, c, i, j, k] + bias[o,i,j]
with k = di*3 + dj (3x3 kernel, stride 1).

Strategy: shard over output rows i across 8 cores (4 rows each). Each core
owns 1/8 of the per-location weight (the dominant traffic) and a 6-row halo
slice of x. Per output location (i,j) the contraction (c,k)=288 is split into
3 chunks of 96 = (c,di) indexed, chunked over dj; each chunk is one matmul
lhsT=[96,64] rhs=[96,128] accumulating into PSUM [64 o, 128 b]. The bias is
folded into the dj=2 chunk as a 97th contraction row against a constant-ones
rhs partition; the ones ride along as a 33rd all-ones x channel so they cost
no extra DMA. Even/odd j use PE column groups 0/1 (tile_position) so two
locations' matmuls overlap in the array. All matmul data is fp16 (fp32
accumulate in PSUM); output is written fp16 and upcast on the host.

The kernel is DMA-bound (~10.3 MB vs ~300 GB/s of per-core DMA throughput,
vs only ~11 us of PE work). Hard-won scheduling constraints:
  - The HWDGE has 8 completion-semaphore lanes assigned round-robin in
    emission order; DMA #k's trigger BLOCKS on its issuing engine until
    DMA #k-8 completes. So: at most 8 early DMAs, emitted so that every
    later trigger's lane predecessor is already done, and out-DMA triggers
    go on Sync (idle after its w triggers) where a blocked trigger can
    never stall PSUM copies or compute.
  - Weights arrive per row (row 0 and row 3 split fully, rows 1-2 fused)
    in exact PE consumption order, so the first matmul triple needs only
    ~1.2 MB and the last 400 KB piece (wc row 3) gates only ~70 matmuls +
    one copy + one 264 KB output transfer.
  - x arrives as one 99-partition slab per output row (partition p=c*3+di
    holds image row r+di), so row r's matmuls never wait on later rows' x.
  - Each PSUM accumulation group is a consecutive wa,wb,wc triple (the
    scheduler mis-tracks interleaved start/stop groups); PSUM->SBUF copies
    alternate Vector/Scalar per group.
"""

import sys

for _p in ("/opt/trn_rl_repo", "/root/.axon_site/_ro/trn_rl_repo"):
    if _p not in sys.path:
        sys.path.append(_p)

import numpy as np

B = 128
C_IN = 32
C_OUT = 64
OH = OW = 32
KH = KW = 3
H = W = 34
N_CORES = 8
RPC = OH // N_CORES          # output rows per core = 4
HALO = RPC + KH - 1          # x rows per core = 6
NPAIR = OW // 2              # j-pairs per row = 16
NGRP = 4                     # j-pairs per psum group
GRPS = NPAIR // NGRP         # psum groups per row = 4

_DT_MM = "float16"           # matmul operand dtype
_DT_OUT = "float16"          # device output dtype

_prog_cache = {}


def _build_program():
    import concourse.tile as tile
    from concourse import bacc, mybir
    from bass_rust import AP

    dt_mm = getattr(mybir.dt, _DT_MM)
    dt_out = getattr(mybir.dt, _DT_OUT)
    f32 = mybir.dt.float32

    nc = bacc.Bacc("TRN2", target_bir_lowering=False, debug=False,
                   num_devices=N_CORES)

    # Per-core DRAM I/O (host pre-sharded / pre-transposed):
    #   x_in  [c=33, h=6, w=34, b=128]  halo slice, b innermost; c=32 is 1.0
    #   w_in  [r=289, i=4, j=32, o=64]  r = dj*96 + c*3 + di; r=288 is bias
    #   out   [p2=128 (par*64+o), i=4, jh=16, b=128] ; j = 2*jh + par
    x_in = nc.dram_tensor("x", [C_IN + 1, HALO, W, B], dt_mm,
                          kind="ExternalInput").ap()
    w_in = nc.dram_tensor("w", [289, RPC, OW, C_OUT], dt_mm,
                          kind="ExternalInput").ap()
    out = nc.dram_tensor("out", [128, RPC, NPAIR, B], dt_out,
                         kind="ExternalOutput").ap()

    HSTR = W * B                # x_in h-row stride (elements)
    CSTR = HALO * W * B         # x_in c stride

    with tile.TileContext(nc) as tc:
        with (
            tc.tile_pool(name="xpool", bufs=1) as xpool,
            tc.tile_pool(name="wpool", bufs=1) as wpool,
            tc.tile_pool(name="opool", bufs=4) as opool,
            tc.tile_pool(name="pspool", bufs=8, space="PSUM") as pspool,
        ):
            xs = [xpool.tile([99, W, B], dt_mm, tag=f"xs{r}",
                             name=f"xs{r}")
                  for r in range(RPC)]
            # row 0 / rows 1-2 (fused) / row 3 weight tiles per dj-chunk
            wa0 = wpool.tile([96, OW, C_OUT], dt_mm, tag="wa0")
            wb0 = wpool.tile([96, OW, C_OUT], dt_mm, tag="wb0")
            wc0 = wpool.tile([97, OW, C_OUT], dt_mm, tag="wc0")
            wam = wpool.tile([96, 2, OW, C_OUT], dt_mm, tag="wam")
            wbm = wpool.tile([96, 2, OW, C_OUT], dt_mm, tag="wbm")
            wcm = wpool.tile([97, 2, OW, C_OUT], dt_mm, tag="wcm")
            wa3 = wpool.tile([96, OW, C_OUT], dt_mm, tag="wa3")
            wb3 = wpool.tile([96, OW, C_OUT], dt_mm, tag="wb3")
            wc3 = wpool.tile([97, OW, C_OUT], dt_mm, tag="wc3")

            def xsrc(r):
                return AP(x_in.tensor, r * HSTR,
                          [(CSTR, C_IN + 1), (HSTR, KH), (1, W * B)])

            # Emission order IS the 8-lane round-robin semaphore assignment:
            # lanes 0-7 go to {xs0, wa0, wb0, wc0, xs1, xs2, xs3, wam}; every
            # later DMA's lane predecessor completes early, so no trigger
            # blocks long enough to starve a queue. Scalar's ring carries x
            # (row-progressive FIFO), Sync's ring carries w then outputs.
            nc.scalar.dma_start(xs[0][:], xsrc(0))           # lane 0
            nc.sync.dma_start(wa0[:], w_in[0:96, 0])         # lane 1
            nc.sync.dma_start(wb0[:], w_in[96:192, 0])       # lane 2
            nc.sync.dma_start(wc0[:], w_in[192:289, 0])      # lane 3
            nc.scalar.dma_start(xs[1][:], xsrc(1))           # lane 4
            nc.scalar.dma_start(xs[2][:], xsrc(2))           # lane 5
            nc.scalar.dma_start(xs[3][:], xsrc(3))           # lane 6
            nc.sync.dma_start(wam[:], w_in[0:96, 1:3])       # lane 7
            nc.sync.dma_start(wbm[:], w_in[96:192, 1:3])     # lane 0*
            nc.sync.dma_start(wcm[:], w_in[192:289, 1:3])    # lane 1*
            nc.sync.dma_start(wa3[:], w_in[0:96, 3])         # lane 2*
            nc.sync.dma_start(wb3[:], w_in[96:192, 3])       # lane 3*
            nc.sync.dma_start(wc3[:], w_in[192:289, 3])      # lane 4*

            wa = [wa0, wam, wam, wa3]
            wb = [wb0, wbm, wbm, wb3]
            wc = [wc0, wcm, wcm, wc3]

            def lhs(t, i, j):
                if t.shape[1:] == (OW, C_OUT):
                    return t[:, j, :]
                return t[:, i - 1, j, :]

            for i in range(RPC):
                for hh in range(2):
                    oh = opool.tile([128, 2 * NGRP, B], dt_out, tag="op")
                    for gg in range(2):
                        g = 2 * hh + gg
                        ps = pspool.tile([128, NGRP, B], f32)
                        for pig in range(NGRP):
                            for par in range(2):
                                j = 2 * (NGRP * g + pig) + par
                                pslice = ps[64 * par:64 * par + 64, pig, :]
                                tp = (0, 64 * par)
                                nc.tensor.matmul(pslice, lhs(wa[i], i, j),
                                                 xs[i][0:96, j, :],
                                                 start=True, stop=False,
                                                 tile_position=tp)
                                nc.tensor.matmul(pslice, lhs(wb[i], i, j),
                                                 xs[i][0:96, j + 1, :],
                                                 start=False, stop=False,
                                                 tile_position=tp)
                                nc.tensor.matmul(pslice, lhs(wc[i], i, j),
                                                 xs[i][0:97, j + 2, :],
                                                 start=False, stop=True,
                                                 tile_position=tp)
                        dst = oh[:, NGRP * gg:NGRP * (gg + 1), :]
                        if g % 2 == 0:
                            nc.vector.tensor_copy(dst, ps[:])
                        else:
                            nc.scalar.copy(dst, ps[:])
                    nc.sync.dma_start(
                        out[:, i, NGRP * 2 * hh:NGRP * 2 * (hh + 1), :],
                        oh[:])

    nc.compile()
    return nc


def _host_prep(x, weight, bias):
    """Full fp32 inputs -> list of per-core input dicts."""
    np_mm = np.dtype(_DT_MM)
    # x: (B, C, H, W) -> (C+1, H, W, B) with an all-ones channel appended
    x_t = np.ascontiguousarray(x.transpose(1, 2, 3, 0)).astype(np_mm)
    x_t = np.concatenate([x_t, np.ones((1, H, W, B), np_mm)], axis=0)
    # w: (O, C, I, J, K) -> [(dj,c,di)=288, i, j, o], bias appended as row 288
    w_r = weight.reshape(C_OUT, C_IN, OH, OW, KH, KW)
    w_t = w_r.transpose(5, 1, 4, 2, 3, 0).reshape(288, OH, OW, C_OUT)
    b_t = bias.transpose(1, 2, 0)[None]                   # (1, I, J, O)
    w_aug = np.concatenate([w_t, b_t], axis=0).astype(np_mm)  # (289, I, J, O)

    in_maps = []
    for m in range(N_CORES):
        r0 = m * RPC
        in_maps.append({
            "x": np.ascontiguousarray(x_t[:, r0:r0 + HALO]),
            "w": np.ascontiguousarray(w_aug[:, r0:r0 + RPC]),
        })
    return in_maps


def _gather(results):
    out_full = np.empty((B, C_OUT, OH, OW), np.float32)
    for m in range(N_CORES):
        r = results[m]["out"].astype(np.float32)          # (128, 4, 16, 128)
        r = r.reshape(2, C_OUT, RPC, NPAIR, B)            # par,o,i,jh,b
        r = r.transpose(4, 1, 2, 3, 0)                    # b,o,i,jh,par
        out_full[:, :, m * RPC:(m + 1) * RPC, :] = r.reshape(B, C_OUT, RPC, OW)
    return out_full


def kernel(x, weight, bias, _trace=False):
    from concourse.bass_utils import run_bass_kernel_spmd

    if "nc" not in _prog_cache:
        _prog_cache["nc"] = _build_program()
    nc = _prog_cache["nc"]

    in_maps = _host_prep(np.asarray(x), np.asarray(weight), np.asarray(bias))
    res = run_bass_kernel_spmd(nc, in_maps, core_ids=list(range(N_CORES)),
                               trace=_trace)
    out = _gather(res.results)
    if _trace:
        _prog_cache["last_result"] = res
    return out


# revision 7
# speedup vs baseline: 1.0166x; 1.0166x over previous
"""Bass/Trainium2 kernel for the decomposed LocallyConnected2d layer.

out[b,o,i,j] = sum_{c,k} x[b, c, i+di, j+dj] * w[o, c, i, j, k] + bias[o,i,j]
with k = di*3 + dj (3x3 kernel, stride 1).

Strategy: shard over output rows i across 8 cores (4 rows each). Each core
owns 1/8 of the per-location weight (the dominant traffic) and a 6-row halo
slice of x. Per output location (i,j) the contraction (c,k)=288 is split into
3 chunks of 96 = (c,di) indexed, chunked over dj; each chunk is one matmul
lhsT=[96,64] rhs=[96,128] accumulating into PSUM [64 o, 128 b]. The bias is
folded into the dj=2 chunk as a 97th contraction row against a constant-ones
rhs partition; the ones ride along as a 33rd all-ones x channel so they cost
no extra DMA. Even/odd j use PE column groups 0/1 (tile_position) so two
locations' matmuls overlap in the array. All matmul data is fp16 (fp32
accumulate in PSUM); output is written fp16 and upcast on the host.

The kernel is DMA-bound (~10.3 MB vs ~300 GB/s of per-core DMA throughput,
vs only ~11 us of PE work). Hard-won scheduling constraints:
  - The HWDGE has 8 completion-semaphore lanes assigned round-robin in
    emission order; DMA #k's trigger BLOCKS on its issuing engine until
    DMA #k-8 completes. So: at most 8 early DMAs, emitted so that every
    later trigger's lane predecessor is already done, and out-DMA triggers
    go on Sync (idle after its w triggers) where a blocked trigger can
    never stall PSUM copies or compute.
  - Weights arrive per row (row 0 and row 3 split fully, rows 1-2 fused)
    in exact PE consumption order, so the first matmul triple needs only
    ~1.2 MB and the last 400 KB piece (wc row 3) gates only ~70 matmuls +
    one copy + one 264 KB output transfer.
  - x arrives as one 99-partition slab per output row (partition p=c*3+di
    holds image row r+di), so row r's matmuls never wait on later rows' x.
  - Each PSUM accumulation group is a consecutive wa,wb,wc triple (the
    scheduler mis-tracks interleaved start/stop groups); PSUM->SBUF copies
    alternate Vector/Scalar per group.
"""

import sys

for _p in ("/opt/trn_rl_repo", "/root/.axon_site/_ro/trn_rl_repo"):
    if _p not in sys.path:
        sys.path.append(_p)

import numpy as np

B = 128
C_IN = 32
C_OUT = 64
OH = OW = 32
KH = KW = 3
H = W = 34
N_CORES = 8
RPC = OH // N_CORES          # output rows per core = 4
HALO = RPC + KH - 1          # x rows per core = 6
NPAIR = OW // 2              # j-pairs per row = 16
NGRP = 4                     # j-pairs per psum group
GRPS = NPAIR // NGRP         # psum groups per row = 4

_DT_MM = "float16"           # matmul operand dtype
_DT_OUT = "float16"          # device output dtype

_prog_cache = {}


def _build_program():
    import concourse.tile as tile
    from concourse import bacc, mybir
    from bass_rust import AP

    dt_mm = getattr(mybir.dt, _DT_MM)
    dt_out = getattr(mybir.dt, _DT_OUT)
    f32 = mybir.dt.float32

    nc = bacc.Bacc("TRN2", target_bir_lowering=False, debug=False,
                   num_devices=N_CORES)

    # Per-core DRAM I/O (host pre-sharded / pre-transposed):
    #   x_in  [c=33, h=6, w=34, b=128]  halo slice, b innermost; c=32 is 1.0
    #   w_in  [r=289, i=4, j=32, o=64]  r = dj*96 + c*3 + di; r=288 is bias
    #   out   [p2=128 (par*64+o), i=4, jh=16, b=128] ; j = 2*jh + par
    x_in = nc.dram_tensor("x", [C_IN + 1, HALO, W, B], dt_mm,
                          kind="ExternalInput").ap()
    w_in = nc.dram_tensor("w", [289, RPC, OW, C_OUT], dt_mm,
                          kind="ExternalInput").ap()
    out = nc.dram_tensor("out", [128, RPC, NPAIR, B], dt_out,
                         kind="ExternalOutput").ap()

    HSTR = W * B                # x_in h-row stride (elements)
    CSTR = HALO * W * B         # x_in c stride

    with tile.TileContext(nc) as tc:
        with (
            tc.tile_pool(name="xpool", bufs=1) as xpool,
            tc.tile_pool(name="wpool", bufs=1) as wpool,
            tc.tile_pool(name="opool", bufs=4) as opool,
            tc.tile_pool(name="pspool", bufs=8, space="PSUM") as pspool,
        ):
            xs = [xpool.tile([99, W, B], dt_mm, tag=f"xs{r}",
                             name=f"xs{r}")
                  for r in range(RPC)]
            # row-pair weight tiles per dj-chunk (2-row pieces keep the
            # per-partition DMA runs at 8KB; 1-row pieces halve the run size
            # and the descriptor bloat stalls the Sync DGE ring)
            wa0 = wpool.tile([96, 2, OW, C_OUT], dt_mm, tag="wa0")
            wb0 = wpool.tile([96, 2, OW, C_OUT], dt_mm, tag="wb0")
            wc0 = wpool.tile([97, 2, OW, C_OUT], dt_mm, tag="wc0")
            wam = wpool.tile([96, 2, OW, C_OUT], dt_mm, tag="wam")
            wbm = wpool.tile([96, 2, OW, C_OUT], dt_mm, tag="wbm")
            wcm = wpool.tile([97, 2, OW, C_OUT], dt_mm, tag="wcm")

            def xsrc(r):
                return AP(x_in.tensor, r * HSTR,
                          [(CSTR, C_IN + 1), (HSTR, KH), (1, W * B)])

            nc.scalar.dma_start(xs[0][:], xsrc(0))
            nc.sync.dma_start(wa0[:], w_in[0:96, 0:2])
            nc.sync.dma_start(wb0[:], w_in[96:192, 0:2])
            nc.sync.dma_start(wc0[:], w_in[192:289, 0:2])
            nc.scalar.dma_start(xs[1][:], xsrc(1))
            nc.scalar.dma_start(xs[2][:], xsrc(2))
            nc.scalar.dma_start(xs[3][:], xsrc(3))
            nc.sync.dma_start(wam[:], w_in[0:96, 2:4])
            nc.sync.dma_start(wbm[:], w_in[96:192, 2:4])
            nc.sync.dma_start(wcm[:], w_in[192:289, 2:4])

            wa = [wa0, wa0, wam, wam]
            wb = [wb0, wb0, wbm, wbm]
            wc = [wc0, wc0, wcm, wcm]

            def lhs(t, i, j):
                return t[:, i % 2, j, :]

            for i in range(RPC):
                for hh in range(2):
                    oh = opool.tile([128, 2 * NGRP, B], dt_out, tag="op")
                    for gg in range(2):
                        g = 2 * hh + gg
                        ps = pspool.tile([128, NGRP, B], f32)
                        for pig in range(NGRP):
                            for par in range(2):
                                j = 2 * (NGRP * g + pig) + par
                                pslice = ps[64 * par:64 * par + 64, pig, :]
                                tp = (0, 64 * par)
                                nc.tensor.matmul(pslice, lhs(wa[i], i, j),
                                                 xs[i][0:96, j, :],
                                                 start=True, stop=False,
                                                 tile_position=tp)
                                nc.tensor.matmul(pslice, lhs(wb[i], i, j),
                                                 xs[i][0:96, j + 1, :],
                                                 start=False, stop=False,
                                                 tile_position=tp)
                                nc.tensor.matmul(pslice, lhs(wc[i], i, j),
                                                 xs[i][0:97, j + 2, :],
                                                 start=False, stop=True,
                                                 tile_position=tp)
                        dst = oh[:, NGRP * gg:NGRP * (gg + 1), :]
                        if g % 2 == 0:
                            nc.vector.tensor_copy(dst, ps[:])
                        else:
                            nc.scalar.copy(dst, ps[:])
                    nc.sync.dma_start(
                        out[:, i, NGRP * 2 * hh:NGRP * 2 * (hh + 1), :],
                        oh[:])

    nc.compile()
    return nc


def _host_prep(x, weight, bias):
    """Full fp32 inputs -> list of per-core input dicts."""
    np_mm = np.dtype(_DT_MM)
    # x: (B, C, H, W) -> (C+1, H, W, B) with an all-ones channel appended
    x_t = np.ascontiguousarray(x.transpose(1, 2, 3, 0)).astype(np_mm)
    x_t = np.concatenate([x_t, np.ones((1, H, W, B), np_mm)], axis=0)
    # w: (O, C, I, J, K) -> [(dj,c,di)=288, i, j, o], bias appended as row 288
    w_r = weight.reshape(C_OUT, C_IN, OH, OW, KH, KW)
    w_t = w_r.transpose(5, 1, 4, 2, 3, 0).reshape(288, OH, OW, C_OUT)
    b_t = bias.transpose(1, 2, 0)[None]                   # (1, I, J, O)
    w_aug = np.concatenate([w_t, b_t], axis=0).astype(np_mm)  # (289, I, J, O)

    in_maps = []
    for m in range(N_CORES):
        r0 = m * RPC
        in_maps.append({
            "x": np.ascontiguousarray(x_t[:, r0:r0 + HALO]),
            "w": np.ascontiguousarray(w_aug[:, r0:r0 + RPC]),
        })
    return in_maps


def _gather(results):
    out_full = np.empty((B, C_OUT, OH, OW), np.float32)
    for m in range(N_CORES):
        r = results[m]["out"].astype(np.float32)          # (128, 4, 16, 128)
        r = r.reshape(2, C_OUT, RPC, NPAIR, B)            # par,o,i,jh,b
        r = r.transpose(4, 1, 2, 3, 0)                    # b,o,i,jh,par
        out_full[:, :, m * RPC:(m + 1) * RPC, :] = r.reshape(B, C_OUT, RPC, OW)
    return out_full


def kernel(x, weight, bias, _trace=False):
    from concourse.bass_utils import run_bass_kernel_spmd

    if "nc" not in _prog_cache:
        _prog_cache["nc"] = _build_program()
    nc = _prog_cache["nc"]

    in_maps = _host_prep(np.asarray(x), np.asarray(weight), np.asarray(bias))
    res = run_bass_kernel_spmd(nc, in_maps, core_ids=list(range(N_CORES)),
                               trace=_trace)
    out = _gather(res.results)
    if _trace:
        _prog_cache["last_result"] = res
    return out
